# revision 15
# baseline (speedup 1.0000x reference)
"""DiagonalSSMLayer Trainium2 kernel.

Full (unsharded) inputs in, full output out. Internally: data-parallel over
batch across 8 NeuronCores (B=8, one batch element per core).

Per-core computation for x_b [S=8192, D=1024]:
    xn    = layernorm(x)
    alpha = sigmoid(xn @ W_a.T + b_a)          # [S, 32]
    b     = xn @ W_in.T + b_in                 # [S, 32]
    h_t   = alpha_t * h_{t-1} + b_t            # scan along S
    out   = x + h @ W_out.T + b_out

Device-side structure (per 512-seq superchunk, 16 of them):
  - DMA x [128, 4, 1024] (2 MB contiguous), loads on the SP HWDGE ring
  - LN stats (mean, var): DVE bn_stats/bn_aggr on 2 of 4 blocks, Pool
    accum-sums on the other 2 (engine balance); gamma/beta are folded into
    the projection weights on the host
  - r = rsqrt(var+eps) via bit-hack + 3 Newton steps on Pool (batched
    [128, 4]) - avoids ACT Sqrt, which lives in a different ACT table set
    than Sigmoid and would thrash table loads
  - y = (x-mu)*r via one ACT Identity op per block (per-partition scale/bias)
  - PE transposes y -> yT [d, seq] via identity matmuls (PSUM), ACT copies
  - fused in-projection matmul chain: G[64, 512] = W_cat @ yT (PSUM accum)
  - alpha = sigmoid(G[0:32] + b~a), b = G[32:64] + b~in  (ACT from PSUM)
  - tensor_tensor_scan (DVE): h = alpha * h_prev + b, chained via initial
  - out-proj: O[128s, 1024d] = [h; ones].T @ [W_out.T; b_out]  (PSUM)
  - DVE: o = O + x (residual), DMA store 2 MB

matmul_dt=float32r runs the projection matmuls in the PE's single-pass fp32
mode (1 cycle/row at N>=512 vs 4 for exact fp32); transposes stay exact fp32.
"""

import sys
from contextlib import ExitStack

if "/opt/trn_rl_repo" not in sys.path:
    sys.path.insert(0, "/opt/trn_rl_repo")

import numpy as np

import concourse.bass as bass
import concourse.bacc as bacc
import concourse.tile as tile
from concourse import mybir
from concourse.bass_utils import run_bass_kernel_spmd

F32 = mybir.dt.float32
F32R = mybir.dt.float32r
I32 = mybir.dt.int32
OP = mybir.AluOpType
AF = mybir.ActivationFunctionType

B, S, D = 8, 8192, 1024
HN = 32          # H * n state channels
K2 = 2 * HN      # alpha + b fused projection output channels
LN_EPS = 1e-5
RSQRT_MAGIC = 0x5F3759DF

SC = 512         # seq superchunk
NSC = S // SC    # 16
NB = SC // 128   # 4 seq blocks of 128 per superchunk
ND = D // 128    # 8 d-slices

_PROGRAM_CACHE = {}


def build_program(matmul_dt=F32R, repeat=1):
    """Build the single-core Bass program (SPMD across the 8 cores).

    repeat>1 re-runs the whole kernel body that many times inside one NEFF
    (used only for timing: t(K) - t(1) cancels dispatch overhead)."""
    nc = bacc.Bacc("TRN2", target_bir_lowering=False, debug=False, num_devices=B)

    x_in = nc.declare_dram_parameter("x", [S, D], F32, isOutput=False)
    # W_cat.T d-sliced: [128(d within slice), 8(d slice), 64(out ch)]
    w_in_d = nc.declare_dram_parameter("w_in", [128, ND, K2], F32, isOutput=False)
    b_t_d = nc.declare_dram_parameter("b_t", [K2, 1], F32, isOutput=False)
    # [W_out.T; b_out]: [33, 1024]
    w_out_d = nc.declare_dram_parameter("w_out", [HN + 1, D], F32, isOutput=False)
    ident_d = nc.declare_dram_parameter("ident", [128, 128], F32, isOutput=False)
    out_d = nc.declare_dram_parameter("out", [S, D], F32, isOutput=True)

    with tile.TileContext(nc) as tc, ExitStack() as ctx:
        consts = ctx.enter_context(tc.tile_pool(name="consts", bufs=1))
        xpool = ctx.enter_context(tc.tile_pool(name="xpool", bufs=2))
        ypool = ctx.enter_context(tc.tile_pool(name="ypool", bufs=3))
        ytpool = ctx.enter_context(tc.tile_pool(name="ytpool", bufs=2))
        stat = ctx.enter_context(tc.tile_pool(name="stat", bufs=3))
        abpool = ctx.enter_context(tc.tile_pool(name="abpool", bufs=3))
        hpool = ctx.enter_context(tc.tile_pool(name="hpool", bufs=3))
        opool = ctx.enter_context(tc.tile_pool(name="opool", bufs=2))
        psum_t = ctx.enter_context(tc.tile_pool(name="psum_t", bufs=3, space="PSUM"))
        psum_g = ctx.enter_context(tc.tile_pool(name="psum_g", bufs=1, space="PSUM"))
        psum_o = ctx.enter_context(tc.tile_pool(name="psum_o", bufs=2, space="PSUM"))

        # ---- constants ----
        w_in_sb = consts.tile([128, ND, K2], F32)
        nc.sync.dma_start(out=w_in_sb, in_=w_in_d[:, :, :])
        w_out_sb = consts.tile([HN + 1, D], F32)
        nc.sync.dma_start(out=w_out_sb, in_=w_out_d[:, :])
        if matmul_dt is not F32:
            w_in_mm = consts.tile([128, ND, K2], matmul_dt)
            nc.vector.tensor_copy(out=w_in_mm, in_=w_in_sb)
            w_out_mm = consts.tile([HN + 1, D], matmul_dt)
            nc.vector.tensor_copy(out=w_out_mm, in_=w_out_sb)
        else:
            w_in_mm, w_out_mm = w_in_sb, w_out_sb
        b_t_sb = consts.tile([K2, 1], F32)
        nc.sync.dma_start(out=b_t_sb, in_=b_t_d[:, :])
        ident = consts.tile([128, 128], F32)
        nc.sync.dma_start(out=ident, in_=ident_d[:, :])
        magic = consts.tile([128, NB], I32)
        nc.gpsimd.memset(magic, RSQRT_MAGIC)
        one_i = consts.tile([128, NB], I32)
        nc.gpsimd.memset(one_i, 1)
        c15 = consts.tile([128, NB], F32)
        nc.gpsimd.memset(c15, 1.5)
        mhalf = consts.tile([128, NB], F32)
        nc.gpsimd.memset(mhalf, -0.5)
        mneg1 = consts.tile([128, NB], F32)
        nc.gpsimd.memset(mneg1, -1.0)
        ceps = consts.tile([128, NB], F32)
        nc.gpsimd.memset(ceps, LN_EPS)

        for _rep in range(repeat):
          h_prev = None
          for sc in range(NSC):
            s0 = sc * SC
            x_t = xpool.tile([128, NB, D], F32)
            nc.sync.dma_start(
                out=x_t,
                in_=x_in[s0 : s0 + SC, :].rearrange("(c p) d -> p c d", p=128),
            )

            # ---- LN stats for the 4 blocks -> mv[:, c, {mu, var}] (DVE) ----
            mv = stat.tile([128, NB, 2], F32, tag="mv")
            for c in range(NB):
                xblk = x_t[:, c, :]
                stats = stat.tile([128, 2, nc.vector.BN_STATS_DIM], F32, tag="bs")
                nc.vector.bn_stats(out=stats[:, 0, :], in_=xblk[:, 0:512])
                nc.vector.bn_stats(out=stats[:, 1, :], in_=xblk[:, 512:1024])
                nc.vector.bn_aggr(out=mv[:, c, :], in_=stats)

            # ---- r = rsqrt(var + eps), mb = -mu * r  (Pool, batched [128,4],
            #      bit-hack seed + 3 Newton steps; Pool only runs TensorTensor) ----
            v4 = stat.tile([128, NB], F32, tag="v4")
            nc.gpsimd.tensor_tensor(out=v4, in0=mv[:, :, 1], in1=ceps, op=OP.add)
            r4 = stat.tile([128, NB], F32, tag="r4")
            t4 = stat.tile([128, NB], F32, tag="t4")
            # seed: bits = MAGIC - (v_bits >> 1); the shift runs on DVE
            # (Pool shift ops require int64 output on trn2)
            nc.vector.tensor_scalar(
                out=t4.bitcast(I32), in0=v4.bitcast(I32), scalar1=1, scalar2=None,
                op0=OP.logical_shift_right,
            )
            nc.gpsimd.tensor_tensor(
                out=r4.bitcast(I32), in0=magic, in1=t4.bitcast(I32), op=OP.subtract
            )
            for _ in range(3):
                nc.gpsimd.tensor_tensor(out=t4, in0=r4, in1=r4, op=OP.mult)
                nc.gpsimd.tensor_tensor(out=t4, in0=t4, in1=v4, op=OP.mult)
                nc.gpsimd.tensor_tensor(out=t4, in0=t4, in1=mhalf, op=OP.mult)
                nc.gpsimd.tensor_tensor(out=t4, in0=t4, in1=c15, op=OP.add)
                nc.gpsimd.tensor_tensor(out=r4, in0=r4, in1=t4, op=OP.mult)
            mb4 = stat.tile([128, NB], F32, tag="mb4")
            nc.gpsimd.tensor_tensor(out=mb4, in0=mv[:, :, 0], in1=r4, op=OP.mult)
            nc.gpsimd.tensor_tensor(out=mb4, in0=mb4, in1=mneg1, op=OP.mult)

            # ---- y = (x - mu) * r; transpose to yT ----
            yt = ytpool.tile([128, ND, SC], matmul_dt)
            for c in range(NB):
                y_t = ypool.tile([128, D], F32)
                nc.scalar.activation(
                    out=y_t, in_=x_t[:, c, :], func=AF.Identity,
                    bias=mb4[:, c : c + 1], scale=r4[:, c : c + 1],
                )
                pt0 = psum_t.tile([128, 512], F32, tag="pt")
                pt1 = psum_t.tile([128, 512], F32, tag="pt")
                for i in range(ND):
                    dst = pt0 if i < 4 else pt1
                    nc.tensor.transpose(
                        dst[:, (i % 4) * 128 : (i % 4 + 1) * 128],
                        y_t[:, i * 128 : (i + 1) * 128],
                        ident,
                    )
                nc.scalar.copy(
                    out=yt[:, 0:4, c * 128 : (c + 1) * 128],
                    in_=pt0.rearrange("p (a b) -> p a b", a=4),
                )
                nc.scalar.copy(
                    out=yt[:, 4:8, c * 128 : (c + 1) * 128],
                    in_=pt1.rearrange("p (a b) -> p a b", a=4),
                )

            # ---- fused input projections: G[64, 512] ----
            g_ps = psum_g.tile([K2, SC], F32)
            for i in range(ND):
                nc.tensor.matmul(
                    g_ps,
                    lhsT=w_in_mm[:, i, :],
                    rhs=yt[:, i, :],
                    start=(i == 0),
                    stop=(i == ND - 1),
                )
            alpha_t = abpool.tile([HN, SC], F32, tag="alpha")
            nc.scalar.activation(
                out=alpha_t, in_=g_ps[0:HN, :], func=AF.Sigmoid,
                bias=b_t_sb[0:HN], scale=1.0,
            )
            bv_t = abpool.tile([HN, SC], F32, tag="bv")
            nc.scalar.activation(
                out=bv_t, in_=g_ps[HN:K2, :], func=AF.Identity,
                bias=b_t_sb[HN:K2], scale=1.0,
            )

            # ---- the recurrence: h_t = alpha_t * h_{t-1} + b_t ----
            h_t = hpool.tile([HN + 1, SC], matmul_dt)
            nc.gpsimd.memset(h_t[HN : HN + 1, :].bitcast(F32), 1.0)
            nc.vector.tensor_tensor_scan(
                out=h_t[0:HN, :],
                data0=alpha_t,
                data1=bv_t,
                initial=0.0 if h_prev is None else h_prev[0:HN, SC - 1 : SC],
                op0=OP.mult,
                op1=OP.add,
            )
            h_prev = h_t

            # ---- output projection + residual ----
            o_sb = opool.tile([128, NB, D], F32)
            for c in range(NB):
                lhs = h_t[:, c * 128 : (c + 1) * 128]
                o_ps = psum_o.tile([128, D], F32, tag="ops")
                for half in range(2):
                    nc.tensor.matmul(
                        o_ps[:, half * 512 : (half + 1) * 512],
                        lhsT=lhs,
                        rhs=w_out_mm[:, half * 512 : (half + 1) * 512],
                        start=True,
                        stop=True,
                    )
                nc.vector.tensor_add(
                    out=o_sb[:, c, :], in0=o_ps, in1=x_t[:, c, :]
                )
            nc.sync.dma_start(
                out=out_d[s0 : s0 + SC, :].rearrange("(c p) d -> p c d", p=128),
                in_=o_sb,
            )

    nc.compile()
    return nc


def _prep_host_inputs(x, W_a, b_a, W_in, b_in, W_out, b_out, ln_gamma, ln_beta):
    """Fold gamma/beta into the projection weights; lay out for the device."""
    f = np.float32
    W_cat = np.concatenate(
        [W_a * ln_gamma[None, :], W_in * ln_gamma[None, :]], axis=0
    ).astype(f)  # [64, 1024]
    w_in_host = np.ascontiguousarray(
        W_cat.T.reshape(ND, 128, K2).transpose(1, 0, 2)
    ).astype(f)  # [128, 8, 64]
    b_t_host = np.concatenate(
        [b_a + W_a @ ln_beta, b_in + W_in @ ln_beta], axis=0
    ).astype(f)[:, None]  # [64, 1]
    w_out_host = np.ascontiguousarray(
        np.concatenate([W_out.T, b_out[None, :]], axis=0)
    ).astype(f)  # [33, 1024]
    ident_host = np.eye(128, dtype=f)
    shared = {
        "w_in": w_in_host,
        "b_t": b_t_host,
        "w_out": w_out_host,
        "ident": ident_host,
    }
    in_maps = [
        {"x": np.ascontiguousarray(x[i]).astype(f), **shared} for i in range(B)
    ]
    return in_maps


def run(inputs, trace=False, matmul_dt=F32R):
    key = str(matmul_dt)
    if key not in _PROGRAM_CACHE:
        _PROGRAM_CACHE[key] = build_program(matmul_dt)
    nc = _PROGRAM_CACHE[key]
    in_maps = _prep_host_inputs(**inputs)
    res = run_bass_kernel_spmd(nc, in_maps, list(range(B)), trace=trace)
    out = np.stack([res.results[i]["out"] for i in range(B)], axis=0)
    return out, res


def kernel(**inputs):
    out, _ = run(inputs)
    return out


# revision 16
# speedup vs baseline: 1.5135x; 1.5135x over previous
"""DiagonalSSMLayer Trainium2 kernel.

Full (unsharded) inputs in, full output out. Internally: data-parallel over
batch across 8 NeuronCores (B=8, one batch element per core).

Per-core computation for x_b [S=8192, D=1024]:
    xn    = layernorm(x)
    alpha = sigmoid(xn @ W_a.T + b_a)          # [S, 32]
    b     = xn @ W_in.T + b_in                 # [S, 32]
    h_t   = alpha_t * h_{t-1} + b_t            # scan along S
    out   = x + h @ W_out.T + b_out

Device-side structure (per 512-seq superchunk, 16 of them):
  - DMA x [128, 4, 1024] (2 MB contiguous), loads on the SP HWDGE ring
  - LN stats (mean, var): DVE bn_stats/bn_aggr on 2 of 4 blocks, Pool
    accum-sums on the other 2 (engine balance); gamma/beta are folded into
    the projection weights on the host
  - r = rsqrt(var+eps) via bit-hack + 3 Newton steps on Pool (batched
    [128, 4]) - avoids ACT Sqrt, which lives in a different ACT table set
    than Sigmoid and would thrash table loads
  - y = (x-mu)*r via one ACT Identity op per block (per-partition scale/bias)
  - PE transposes y -> yT [d, seq] via identity matmuls (PSUM), ACT copies
  - fused in-projection matmul chain: G[64, 512] = W_cat @ yT (PSUM accum)
  - alpha = sigmoid(G[0:32] + b~a), b = G[32:64] + b~in  (ACT from PSUM)
  - tensor_tensor_scan (DVE): h = alpha * h_prev + b, chained via initial
  - out-proj: O[128s, 1024d] = [h; ones].T @ [W_out.T; b_out]  (PSUM)
  - DVE: o = O + x (residual), DMA store 2 MB

matmul_dt=float32r runs the projection matmuls in the PE's single-pass fp32
mode (1 cycle/row at N>=512 vs 4 for exact fp32); transposes stay exact fp32.
"""

import sys
from contextlib import ExitStack

if "/opt/trn_rl_repo" not in sys.path:
    sys.path.insert(0, "/opt/trn_rl_repo")

import numpy as np

import concourse.bass as bass
import concourse.bacc as bacc
import concourse.tile as tile
from concourse import mybir
from concourse.bass_utils import run_bass_kernel_spmd

F32 = mybir.dt.float32
F32R = mybir.dt.float32r
I32 = mybir.dt.int32
OP = mybir.AluOpType
AF = mybir.ActivationFunctionType

B, S, D = 8, 8192, 1024
HN = 32          # H * n state channels
K2 = 2 * HN      # alpha + b fused projection output channels
LN_EPS = 1e-5
RSQRT_MAGIC = 0x5F3759DF

SC = 512         # seq superchunk
NSC = S // SC    # 16
NB = SC // 128   # 4 seq blocks of 128 per superchunk
ND = D // 128    # 8 d-slices

_PROGRAM_CACHE = {}


def build_program(matmul_dt=F32R, repeat=1, variant="full"):
    """Build the single-core Bass program (SPMD across the 8 cores).

    repeat>1 re-runs the whole kernel body that many times inside one NEFF
    (used only for timing: t(K) - t(1) cancels dispatch overhead).
    variant="dma" is a timing ablation: just load/store each superchunk."""
    nc = bacc.Bacc("TRN2", target_bir_lowering=False, debug=False, num_devices=B)

    x_in = nc.declare_dram_parameter("x", [S, D], F32, isOutput=False)
    # W_cat.T d-sliced: [128(d within slice), 8(d slice), 64(out ch)]
    w_in_d = nc.declare_dram_parameter("w_in", [128, ND, K2], F32, isOutput=False)
    b_t_d = nc.declare_dram_parameter("b_t", [K2, 1], F32, isOutput=False)
    # [W_out.T; b_out]: [33, 1024]
    w_out_d = nc.declare_dram_parameter("w_out", [HN + 1, D], F32, isOutput=False)
    ident_d = nc.declare_dram_parameter("ident", [128, 128], F32, isOutput=False)
    out_d = nc.declare_dram_parameter("out", [S, D], F32, isOutput=True)

    with tile.TileContext(nc) as tc, ExitStack() as ctx:
        consts = ctx.enter_context(tc.tile_pool(name="consts", bufs=1))
        xpool = ctx.enter_context(tc.tile_pool(name="xpool", bufs=2))
        ypool = ctx.enter_context(tc.tile_pool(name="ypool", bufs=3))
        ytpool = ctx.enter_context(tc.tile_pool(name="ytpool", bufs=2))
        stat = ctx.enter_context(tc.tile_pool(name="stat", bufs=3))
        abpool = ctx.enter_context(tc.tile_pool(name="abpool", bufs=3))
        hpool = ctx.enter_context(tc.tile_pool(name="hpool", bufs=3))
        opool = ctx.enter_context(tc.tile_pool(name="opool", bufs=2))
        psum_t = ctx.enter_context(tc.tile_pool(name="psum_t", bufs=3, space="PSUM"))
        psum_g = ctx.enter_context(tc.tile_pool(name="psum_g", bufs=1, space="PSUM"))
        psum_o = ctx.enter_context(tc.tile_pool(name="psum_o", bufs=2, space="PSUM"))

        # ---- constants ----
        w_in_sb = consts.tile([128, ND, K2], F32)
        nc.sync.dma_start(out=w_in_sb, in_=w_in_d[:, :, :])
        w_out_sb = consts.tile([HN + 1, D], F32)
        nc.sync.dma_start(out=w_out_sb, in_=w_out_d[:, :])
        if matmul_dt is not F32:
            w_in_mm = consts.tile([128, ND, K2], matmul_dt)
            nc.vector.tensor_copy(out=w_in_mm, in_=w_in_sb)
            w_out_mm = consts.tile([HN + 1, D], matmul_dt)
            nc.vector.tensor_copy(out=w_out_mm, in_=w_out_sb)
        else:
            w_in_mm, w_out_mm = w_in_sb, w_out_sb
        b_t_sb = consts.tile([K2, 1], F32)
        nc.sync.dma_start(out=b_t_sb, in_=b_t_d[:, :])
        ident = consts.tile([128, 128], F32)
        nc.sync.dma_start(out=ident, in_=ident_d[:, :])
        magic = consts.tile([128, NB], I32)
        nc.gpsimd.memset(magic, RSQRT_MAGIC)
        one_i = consts.tile([128, NB], I32)
        nc.gpsimd.memset(one_i, 1)
        c15 = consts.tile([128, NB], F32)
        nc.gpsimd.memset(c15, 1.5)
        mhalf = consts.tile([128, NB], F32)
        nc.gpsimd.memset(mhalf, -0.5)
        mneg1 = consts.tile([128, NB], F32)
        nc.gpsimd.memset(mneg1, -1.0)
        ceps = consts.tile([128, NB], F32)
        nc.gpsimd.memset(ceps, LN_EPS)

        for _rep in range(repeat):
          h_prev = None
          for sc in range(NSC):
            s0 = sc * SC
            x_t = xpool.tile([128, NB, D], F32)
            nc.sync.dma_start(
                out=x_t,
                in_=x_in[s0 : s0 + SC, :].rearrange("(c p) d -> p c d", p=128),
            )
            if variant == "dma":
                nc.sync.dma_start(
                    out=out_d[s0 : s0 + SC, :].rearrange("(c p) d -> p c d", p=128),
                    in_=x_t,
                )
                continue

            # ---- LN stats for the 4 blocks -> mv[:, c, {mu, var}] (DVE) ----
            mv = stat.tile([128, NB, 2], F32, tag="mv")
            for c in range(NB):
                xblk = x_t[:, c, :]
                stats = stat.tile([128, 2, nc.vector.BN_STATS_DIM], F32, tag="bs")
                nc.vector.bn_stats(out=stats[:, 0, :], in_=xblk[:, 0:512])
                nc.vector.bn_stats(out=stats[:, 1, :], in_=xblk[:, 512:1024])
                nc.vector.bn_aggr(out=mv[:, c, :], in_=stats)

            # ---- r = rsqrt(var + eps), mb = -mu * r  (Pool, batched [128,4],
            #      bit-hack seed + 3 Newton steps; Pool only runs TensorTensor) ----
            v4 = stat.tile([128, NB], F32, tag="v4")
            nc.gpsimd.tensor_tensor(out=v4, in0=mv[:, :, 1], in1=ceps, op=OP.add)
            r4 = stat.tile([128, NB], F32, tag="r4")
            t4 = stat.tile([128, NB], F32, tag="t4")
            # seed: bits = MAGIC - (v_bits >> 1); the shift runs on DVE
            # (Pool shift ops require int64 output on trn2)
            nc.vector.tensor_scalar(
                out=t4.bitcast(I32), in0=v4.bitcast(I32), scalar1=1, scalar2=None,
                op0=OP.logical_shift_right,
            )
            nc.gpsimd.tensor_tensor(
                out=r4.bitcast(I32), in0=magic, in1=t4.bitcast(I32), op=OP.subtract
            )
            for _ in range(3):
                nc.gpsimd.tensor_tensor(out=t4, in0=r4, in1=r4, op=OP.mult)
                nc.gpsimd.tensor_tensor(out=t4, in0=t4, in1=v4, op=OP.mult)
                nc.gpsimd.tensor_tensor(out=t4, in0=t4, in1=mhalf, op=OP.mult)
                nc.gpsimd.tensor_tensor(out=t4, in0=t4, in1=c15, op=OP.add)
                nc.gpsimd.tensor_tensor(out=r4, in0=r4, in1=t4, op=OP.mult)
            mb4 = stat.tile([128, NB], F32, tag="mb4")
            nc.gpsimd.tensor_tensor(out=mb4, in0=mv[:, :, 0], in1=r4, op=OP.mult)
            nc.gpsimd.tensor_tensor(out=mb4, in0=mb4, in1=mneg1, op=OP.mult)

            # ---- y = (x - mu) * r; transpose to yT ----
            yt = ytpool.tile([128, ND, SC], matmul_dt)
            for c in range(NB):
                y_t = ypool.tile([128, D], F32)
                nc.scalar.activation(
                    out=y_t, in_=x_t[:, c, :], func=AF.Identity,
                    bias=mb4[:, c : c + 1], scale=r4[:, c : c + 1],
                )
                pt0 = psum_t.tile([128, 512], F32, tag="pt")
                pt1 = psum_t.tile([128, 512], F32, tag="pt")
                for i in range(ND):
                    dst = pt0 if i < 4 else pt1
                    nc.tensor.transpose(
                        dst[:, (i % 4) * 128 : (i % 4 + 1) * 128],
                        y_t[:, i * 128 : (i + 1) * 128],
                        ident,
                    )
                nc.scalar.copy(
                    out=yt[:, 0:4, c * 128 : (c + 1) * 128],
                    in_=pt0.rearrange("p (a b) -> p a b", a=4),
                )
                nc.scalar.copy(
                    out=yt[:, 4:8, c * 128 : (c + 1) * 128],
                    in_=pt1.rearrange("p (a b) -> p a b", a=4),
                )

            # ---- fused input projections: G[64, 512] ----
            g_ps = psum_g.tile([K2, SC], F32)
            for i in range(ND):
                nc.tensor.matmul(
                    g_ps,
                    lhsT=w_in_mm[:, i, :],
                    rhs=yt[:, i, :],
                    start=(i == 0),
                    stop=(i == ND - 1),
                )
            alpha_t = abpool.tile([HN, SC], F32, tag="alpha")
            nc.scalar.activation(
                out=alpha_t, in_=g_ps[0:HN, :], func=AF.Sigmoid,
                bias=b_t_sb[0:HN], scale=1.0,
            )
            bv_t = abpool.tile([HN, SC], F32, tag="bv")
            nc.scalar.activation(
                out=bv_t, in_=g_ps[HN:K2, :], func=AF.Identity,
                bias=b_t_sb[HN:K2], scale=1.0,
            )

            # ---- the recurrence: h_t = alpha_t * h_{t-1} + b_t ----
            h_t = hpool.tile([HN + 1, SC], matmul_dt)
            nc.gpsimd.memset(h_t[HN : HN + 1, :].bitcast(F32), 1.0)
            nc.vector.tensor_tensor_scan(
                out=h_t[0:HN, :],
                data0=alpha_t,
                data1=bv_t,
                initial=0.0 if h_prev is None else h_prev[0:HN, SC - 1 : SC],
                op0=OP.mult,
                op1=OP.add,
            )
            h_prev = h_t

            # ---- output projection + residual ----
            o_sb = opool.tile([128, NB, D], F32)
            for c in range(NB):
                lhs = h_t[:, c * 128 : (c + 1) * 128]
                o_ps = psum_o.tile([128, D], F32, tag="ops")
                for half in range(2):
                    nc.tensor.matmul(
                        o_ps[:, half * 512 : (half + 1) * 512],
                        lhsT=lhs,
                        rhs=w_out_mm[:, half * 512 : (half + 1) * 512],
                        start=True,
                        stop=True,
                    )
                nc.vector.tensor_add(
                    out=o_sb[:, c, :], in0=o_ps, in1=x_t[:, c, :]
                )
            nc.sync.dma_start(
                out=out_d[s0 : s0 + SC, :].rearrange("(c p) d -> p c d", p=128),
                in_=o_sb,
            )

    nc.compile()
    return nc


def _prep_host_inputs(x, W_a, b_a, W_in, b_in, W_out, b_out, ln_gamma, ln_beta):
    """Fold gamma/beta into the projection weights; lay out for the device."""
    f = np.float32
    W_cat = np.concatenate(
        [W_a * ln_gamma[None, :], W_in * ln_gamma[None, :]], axis=0
    ).astype(f)  # [64, 1024]
    w_in_host = np.ascontiguousarray(
        W_cat.T.reshape(ND, 128, K2).transpose(1, 0, 2)
    ).astype(f)  # [128, 8, 64]
    b_t_host = np.concatenate(
        [b_a + W_a @ ln_beta, b_in + W_in @ ln_beta], axis=0
    ).astype(f)[:, None]  # [64, 1]
    w_out_host = np.ascontiguousarray(
        np.concatenate([W_out.T, b_out[None, :]], axis=0)
    ).astype(f)  # [33, 1024]
    ident_host = np.eye(128, dtype=f)
    shared = {
        "w_in": w_in_host,
        "b_t": b_t_host,
        "w_out": w_out_host,
        "ident": ident_host,
    }
    in_maps = [
        {"x": np.ascontiguousarray(x[i]).astype(f), **shared} for i in range(B)
    ]
    return in_maps


def run(inputs, trace=False, matmul_dt=F32R):
    key = str(matmul_dt)
    if key not in _PROGRAM_CACHE:
        _PROGRAM_CACHE[key] = build_program(matmul_dt)
    nc = _PROGRAM_CACHE[key]
    in_maps = _prep_host_inputs(**inputs)
    res = run_bass_kernel_spmd(nc, in_maps, list(range(B)), trace=trace)
    out = np.stack([res.results[i]["out"] for i in range(B)], axis=0)
    return out, res


def kernel(**inputs):
    out, _ = run(inputs)
    return out


# revision 17
# speedup vs baseline: 4.2608x; 2.8153x over previous
"""DiagonalSSMLayer Trainium2 kernel.

Full (unsharded) inputs in, full output out. Internally: data-parallel over
batch across 8 NeuronCores (B=8, one batch element per core).

Per-core computation for x_b [S=8192, D=1024]:
    xn    = layernorm(x)
    alpha = sigmoid(xn @ W_a.T + b_a)          # [S, 32]
    b     = xn @ W_in.T + b_in                 # [S, 32]
    h_t   = alpha_t * h_{t-1} + b_t            # scan along S
    out   = x + h @ W_out.T + b_out

Device-side structure (per 512-seq superchunk, 16 of them):
  - DMA x [128, 4, 1024] (2 MB contiguous), loads on the SP HWDGE ring
  - LN stats (mean, var): DVE bn_stats/bn_aggr on 2 of 4 blocks, Pool
    accum-sums on the other 2 (engine balance); gamma/beta are folded into
    the projection weights on the host
  - r = rsqrt(var+eps) via bit-hack + 3 Newton steps on Pool (batched
    [128, 4]) - avoids ACT Sqrt, which lives in a different ACT table set
    than Sigmoid and would thrash table loads
  - y = (x-mu)*r via one ACT Identity op per block (per-partition scale/bias)
  - PE transposes y -> yT [d, seq] via identity matmuls (PSUM), ACT copies
  - fused in-projection matmul chain: G[64, 512] = W_cat @ yT (PSUM accum)
  - alpha = sigmoid(G[0:32] + b~a), b = G[32:64] + b~in  (ACT from PSUM)
  - tensor_tensor_scan (DVE): h = alpha * h_prev + b, chained via initial
  - out-proj: O[128s, 1024d] = [h; ones].T @ [W_out.T; b_out]  (PSUM)
  - DVE: o = O + x (residual), DMA store 2 MB

matmul_dt=float32r runs the projection matmuls in the PE's single-pass fp32
mode (1 cycle/row at N>=512 vs 4 for exact fp32); transposes stay exact fp32.
"""

import sys
from contextlib import ExitStack

if "/opt/trn_rl_repo" not in sys.path:
    sys.path.insert(0, "/opt/trn_rl_repo")

import numpy as np

import concourse.bass as bass
import concourse.bacc as bacc
import concourse.tile as tile
from concourse import mybir
from concourse.bass_utils import run_bass_kernel_spmd

F32 = mybir.dt.float32
F32R = mybir.dt.float32r
I32 = mybir.dt.int32
OP = mybir.AluOpType
AF = mybir.ActivationFunctionType

B, S, D = 8, 8192, 1024
HN = 32          # H * n state channels
K2 = 2 * HN      # alpha + b fused projection output channels
LN_EPS = 1e-5
RSQRT_MAGIC = 0x5F3759DF

SC = 512         # seq superchunk
NSC = S // SC    # 16
NB = SC // 128   # 4 seq blocks of 128 per superchunk
ND = D // 128    # 8 d-slices

_PROGRAM_CACHE = {}


def build_program(matmul_dt=F32R, repeat=1, variant="full"):
    """Build the single-core Bass program (SPMD across the 8 cores).

    repeat>1 re-runs the whole kernel body that many times inside one NEFF
    (used only for timing: t(K) - t(1) cancels dispatch overhead).
    variant="dma" is a timing ablation: just load/store each superchunk."""
    nc = bacc.Bacc("TRN2", target_bir_lowering=False, debug=False, num_devices=B)

    x_in = nc.declare_dram_parameter("x", [S, D], F32, isOutput=False)
    # W_cat.T d-sliced: [128(d within slice), 8(d slice), 64(out ch)]
    w_in_d = nc.declare_dram_parameter("w_in", [128, ND, K2], F32, isOutput=False)
    b_t_d = nc.declare_dram_parameter("b_t", [K2, 1], F32, isOutput=False)
    # [W_out.T; b_out]: [33, 1024]
    w_out_d = nc.declare_dram_parameter("w_out", [HN + 1, D], F32, isOutput=False)
    ident_d = nc.declare_dram_parameter("ident", [128, 128], F32, isOutput=False)
    out_d = nc.declare_dram_parameter("out", [S, D], F32, isOutput=True)

    with tile.TileContext(nc) as tc, ExitStack() as ctx:
        consts = ctx.enter_context(tc.tile_pool(name="consts", bufs=1))
        xpool = ctx.enter_context(tc.tile_pool(name="xpool", bufs=2))
        ypool = ctx.enter_context(tc.tile_pool(name="ypool", bufs=3))
        ytpool = ctx.enter_context(tc.tile_pool(name="ytpool", bufs=2))
        stat = ctx.enter_context(tc.tile_pool(name="stat", bufs=3))
        abpool = ctx.enter_context(tc.tile_pool(name="abpool", bufs=3))
        hpool = ctx.enter_context(tc.tile_pool(name="hpool", bufs=3))
        opool = ctx.enter_context(tc.tile_pool(name="opool", bufs=2))
        psum_t = ctx.enter_context(tc.tile_pool(name="psum_t", bufs=3, space="PSUM"))
        psum_g = ctx.enter_context(tc.tile_pool(name="psum_g", bufs=1, space="PSUM"))
        psum_o = ctx.enter_context(tc.tile_pool(name="psum_o", bufs=2, space="PSUM"))

        # ---- constants ----
        w_in_sb = consts.tile([128, ND, K2], F32)
        nc.sync.dma_start(out=w_in_sb, in_=w_in_d[:, :, :])
        w_out_sb = consts.tile([HN + 1, D], F32)
        nc.sync.dma_start(out=w_out_sb, in_=w_out_d[:, :])
        if matmul_dt is not F32:
            w_in_mm = consts.tile([128, ND, K2], matmul_dt)
            nc.vector.tensor_copy(out=w_in_mm, in_=w_in_sb)
            w_out_mm = consts.tile([HN + 1, D], matmul_dt)
            nc.vector.tensor_copy(out=w_out_mm, in_=w_out_sb)
        else:
            w_in_mm, w_out_mm = w_in_sb, w_out_sb
        b_t_sb = consts.tile([K2, 1], F32)
        nc.sync.dma_start(out=b_t_sb, in_=b_t_d[:, :])
        ident = consts.tile([128, 128], F32)
        nc.sync.dma_start(out=ident, in_=ident_d[:, :])
        magic = consts.tile([128, NB], I32)
        nc.gpsimd.memset(magic, RSQRT_MAGIC)
        one_i = consts.tile([128, NB], I32)
        nc.gpsimd.memset(one_i, 1)
        c15 = consts.tile([128, NB], F32)
        nc.gpsimd.memset(c15, 1.5)
        mhalf = consts.tile([128, NB], F32)
        nc.gpsimd.memset(mhalf, -0.5)
        mneg1 = consts.tile([128, NB], F32)
        nc.gpsimd.memset(mneg1, -1.0)
        ceps = consts.tile([128, NB], F32)
        nc.gpsimd.memset(ceps, LN_EPS)

        for _rep in range(repeat):
          h_prev = None
          for sc in range(NSC):
            s0 = sc * SC
            x_t = xpool.tile([128, NB, D], F32)
            nc.sync.dma_start(
                out=x_t,
                in_=x_in[s0 : s0 + SC, :].rearrange("(c p) d -> p c d", p=128),
            )
            if variant == "dma":
                nc.sync.dma_start(
                    out=out_d[s0 : s0 + SC, :].rearrange("(c p) d -> p c d", p=128),
                    in_=x_t,
                )
                continue
            if variant == "dma2":
                nc.scalar.dma_start(
                    out=out_d[s0 : s0 + SC, :].rearrange("(c p) d -> p c d", p=128),
                    in_=x_t,
                )
                continue

            # ---- LN stats for the 4 blocks -> mv[:, c, {mu, var}] (DVE) ----
            mv = stat.tile([128, NB, 2], F32, tag="mv")
            for c in range(NB):
                xblk = x_t[:, c, :]
                stats = stat.tile([128, 2, nc.vector.BN_STATS_DIM], F32, tag="bs")
                nc.vector.bn_stats(out=stats[:, 0, :], in_=xblk[:, 0:512])
                nc.vector.bn_stats(out=stats[:, 1, :], in_=xblk[:, 512:1024])
                nc.vector.bn_aggr(out=mv[:, c, :], in_=stats)

            # ---- r = rsqrt(var + eps), mb = -mu * r  (Pool, batched [128,4],
            #      bit-hack seed + 3 Newton steps; Pool only runs TensorTensor) ----
            v4 = stat.tile([128, NB], F32, tag="v4")
            nc.gpsimd.tensor_tensor(out=v4, in0=mv[:, :, 1], in1=ceps, op=OP.add)
            r4 = stat.tile([128, NB], F32, tag="r4")
            t4 = stat.tile([128, NB], F32, tag="t4")
            # seed: bits = MAGIC - (v_bits >> 1); the shift runs on DVE
            # (Pool shift ops require int64 output on trn2)
            nc.vector.tensor_scalar(
                out=t4.bitcast(I32), in0=v4.bitcast(I32), scalar1=1, scalar2=None,
                op0=OP.logical_shift_right,
            )
            nc.gpsimd.tensor_tensor(
                out=r4.bitcast(I32), in0=magic, in1=t4.bitcast(I32), op=OP.subtract
            )
            for _ in range(3):
                nc.gpsimd.tensor_tensor(out=t4, in0=r4, in1=r4, op=OP.mult)
                nc.gpsimd.tensor_tensor(out=t4, in0=t4, in1=v4, op=OP.mult)
                nc.gpsimd.tensor_tensor(out=t4, in0=t4, in1=mhalf, op=OP.mult)
                nc.gpsimd.tensor_tensor(out=t4, in0=t4, in1=c15, op=OP.add)
                nc.gpsimd.tensor_tensor(out=r4, in0=r4, in1=t4, op=OP.mult)
            mb4 = stat.tile([128, NB], F32, tag="mb4")
            nc.gpsimd.tensor_tensor(out=mb4, in0=mv[:, :, 0], in1=r4, op=OP.mult)
            nc.gpsimd.tensor_tensor(out=mb4, in0=mb4, in1=mneg1, op=OP.mult)

            # ---- y = (x - mu) * r; transpose to yT ----
            yt = ytpool.tile([128, ND, SC], matmul_dt)
            for c in range(NB):
                y_t = ypool.tile([128, D], F32)
                nc.scalar.activation(
                    out=y_t, in_=x_t[:, c, :], func=AF.Identity,
                    bias=mb4[:, c : c + 1], scale=r4[:, c : c + 1],
                )
                pt0 = psum_t.tile([128, 512], F32, tag="pt")
                pt1 = psum_t.tile([128, 512], F32, tag="pt")
                for i in range(ND):
                    dst = pt0 if i < 4 else pt1
                    nc.tensor.transpose(
                        dst[:, (i % 4) * 128 : (i % 4 + 1) * 128],
                        y_t[:, i * 128 : (i + 1) * 128],
                        ident,
                    )
                nc.scalar.copy(
                    out=yt[:, 0:4, c * 128 : (c + 1) * 128],
                    in_=pt0.rearrange("p (a b) -> p a b", a=4),
                )
                nc.scalar.copy(
                    out=yt[:, 4:8, c * 128 : (c + 1) * 128],
                    in_=pt1.rearrange("p (a b) -> p a b", a=4),
                )

            # ---- fused input projections: G[64, 512] ----
            g_ps = psum_g.tile([K2, SC], F32)
            for i in range(ND):
                nc.tensor.matmul(
                    g_ps,
                    lhsT=w_in_mm[:, i, :],
                    rhs=yt[:, i, :],
                    start=(i == 0),
                    stop=(i == ND - 1),
                )
            alpha_t = abpool.tile([HN, SC], F32, tag="alpha")
            nc.scalar.activation(
                out=alpha_t, in_=g_ps[0:HN, :], func=AF.Sigmoid,
                bias=b_t_sb[0:HN], scale=1.0,
            )
            bv_t = abpool.tile([HN, SC], F32, tag="bv")
            nc.scalar.activation(
                out=bv_t, in_=g_ps[HN:K2, :], func=AF.Identity,
                bias=b_t_sb[HN:K2], scale=1.0,
            )

            # ---- the recurrence: h_t = alpha_t * h_{t-1} + b_t ----
            h_t = hpool.tile([HN + 1, SC], matmul_dt)
            nc.gpsimd.memset(h_t[HN : HN + 1, :].bitcast(F32), 1.0)
            nc.vector.tensor_tensor_scan(
                out=h_t[0:HN, :],
                data0=alpha_t,
                data1=bv_t,
                initial=0.0 if h_prev is None else h_prev[0:HN, SC - 1 : SC],
                op0=OP.mult,
                op1=OP.add,
            )
            h_prev = h_t

            # ---- output projection + residual ----
            o_sb = opool.tile([128, NB, D], F32)
            for c in range(NB):
                lhs = h_t[:, c * 128 : (c + 1) * 128]
                o_ps = psum_o.tile([128, D], F32, tag="ops")
                for half in range(2):
                    nc.tensor.matmul(
                        o_ps[:, half * 512 : (half + 1) * 512],
                        lhsT=lhs,
                        rhs=w_out_mm[:, half * 512 : (half + 1) * 512],
                        start=True,
                        stop=True,
                    )
                nc.vector.tensor_add(
                    out=o_sb[:, c, :], in0=o_ps, in1=x_t[:, c, :]
                )
            nc.sync.dma_start(
                out=out_d[s0 : s0 + SC, :].rearrange("(c p) d -> p c d", p=128),
                in_=o_sb,
            )

    nc.compile()
    return nc


def _prep_host_inputs(x, W_a, b_a, W_in, b_in, W_out, b_out, ln_gamma, ln_beta):
    """Fold gamma/beta into the projection weights; lay out for the device."""
    f = np.float32
    W_cat = np.concatenate(
        [W_a * ln_gamma[None, :], W_in * ln_gamma[None, :]], axis=0
    ).astype(f)  # [64, 1024]
    w_in_host = np.ascontiguousarray(
        W_cat.T.reshape(ND, 128, K2).transpose(1, 0, 2)
    ).astype(f)  # [128, 8, 64]
    b_t_host = np.concatenate(
        [b_a + W_a @ ln_beta, b_in + W_in @ ln_beta], axis=0
    ).astype(f)[:, None]  # [64, 1]
    w_out_host = np.ascontiguousarray(
        np.concatenate([W_out.T, b_out[None, :]], axis=0)
    ).astype(f)  # [33, 1024]
    ident_host = np.eye(128, dtype=f)
    shared = {
        "w_in": w_in_host,
        "b_t": b_t_host,
        "w_out": w_out_host,
        "ident": ident_host,
    }
    in_maps = [
        {"x": np.ascontiguousarray(x[i]).astype(f), **shared} for i in range(B)
    ]
    return in_maps


def run(inputs, trace=False, matmul_dt=F32R):
    key = str(matmul_dt)
    if key not in _PROGRAM_CACHE:
        _PROGRAM_CACHE[key] = build_program(matmul_dt)
    nc = _PROGRAM_CACHE[key]
    in_maps = _prep_host_inputs(**inputs)
    res = run_bass_kernel_spmd(nc, in_maps, list(range(B)), trace=trace)
    out = np.stack([res.results[i]["out"] for i in range(B)], axis=0)
    return out, res


def kernel(**inputs):
    out, _ = run(inputs)
    return out
